# revision 23
# baseline (speedup 1.0000x reference)
"""Trainium2 Bass kernel for a Qwen2-VL vision transformer block.

Strategy: 8-way sequence-parallel across NeuronCores. Each core owns a
256-row shard of the 2048-token sequence and the full weights (bf16).
K and V for the full sequence are exchanged with two fp8 AllGathers
(K first so scores can start as early as possible, V second so the
AV matmuls catch up while the first heads run). All matmuls run on
the PE with fp32 PSUM accumulation; LayerNorm statistics, the softmax
normalizer and residual adds stay in fp32; RoPE runs in bf16 on the
DVE (2x packed mode).

Schedule notes (vs the naive phase-serial version):
  - DMA queues are specialized: Sync (HWDGE) streams all weights in a
    few large transfers; GpSimd (SWDGE) carries activations, the K/V
    publishes + gathered reloads and the per-head softmax-normalizer
    partition broadcasts.  ScalarE issues NO DMAs so ACTIVATE (exp,
    gelu) never queues behind a transfer.
  - The attention loop is software-pipelined by hand: scores(h) is
    emitted before AV(h-1) so the PE always has a ready matmul while
    ScalarE exponentiates, keeping the PE HAM-warm.
  - The softmax denominator is accumulated by an extra all-ones column
    on the stationary V operand; the per-query 1/Z broadcast across
    partitions uses a GpSimd partition_broadcast (no PE/DMA round trip).
  - The MLP streams W1/W2 in 2-block (655 KB) chunks, software-
    pipelined fc1(j+1) ahead of fc2(j), with quick-gelu batched per
    pair and folded into the weights on the host.
"""

import sys

import numpy as np

for _p in ("/opt/trn_rl_repo",):
    if _p not in sys.path:
        sys.path.insert(0, _p)

import ml_dtypes  # noqa: E402


BF = ml_dtypes.bfloat16

B, S, H = 1, 2048, 1280
NH, HD = 16, 80
MLP = 5120
EPS = 1e-6
NCORES = 8
SL = S // NCORES            # 256 sequence rows per core
SB = SL // 128              # 2 partition blocks per core
HC = H // 128               # 10 contraction chunks over H
MC = MLP // 128             # 40 blocks over the MLP dim
MP = MC // 2                # 20 block-pairs in the MLP stream
KB = S // 128               # 16 key blocks over the full sequence
NCOLS = ((0, 512), (512, 512), (1024, 256))
VCOLS = ((0, 6), (6, 6), (12, 4))     # head-aligned col groups (h0, nh)
SCALE = 1.0 / float(np.sqrt(np.float32(HD)))
HDA = HD + 17               # V augmented: pad + ones col at 96 -> Z row
ZR = 96                     # Z lands on a 32-aligned partition
KT_E = HD * NH * SL         # gathered K^T elements per rank (fp8)
VT_E = SL * NH * HDA        # gathered V-aug elements per rank (fp8)


def _build_bass(use_bias):
    import bass_rust
    import concourse.bacc as bacc
    import concourse.tile as tile
    from concourse import mybir
    from concourse.masks import make_identity

    F32 = mybir.dt.float32
    BF16 = mybir.dt.bfloat16
    FP8 = mybir.dt.float8e4
    AF = mybir.ActivationFunctionType
    OP = mybir.AluOpType

    nc = bacc.Bacc("TRN2", target_bir_lowering=False, debug=False,
                   num_devices=NCORES)

    x_io = nc.dram_tensor("x_loc", [SL, H], F32, kind="ExternalInput")
    cos_io = nc.dram_tensor("cosr", [SL, H], BF16, kind="ExternalInput")
    sin_io = nc.dram_tensor("sins", [SL, H], BF16, kind="ExternalInput")
    wqt_io = nc.dram_tensor("wqt", [H, H], BF16, kind="ExternalInput")
    wkt_io = nc.dram_tensor("wkt", [H, H], BF16, kind="ExternalInput")
    wvt_io = nc.dram_tensor("wvt", [H, H], BF16, kind="ExternalInput")
    wot_io = nc.dram_tensor("wot", [H, H], BF16, kind="ExternalInput")
    w1b_io = nc.dram_tensor("w1b", [MP, 128, 2 * HC * 128], BF16,
                            kind="ExternalInput")
    w2t_io = nc.dram_tensor("w2t", [MP, 128, 2 * H], BF16,
                            kind="ExternalInput")
    bias5_io = nc.dram_tensor("bias5", [5, H], BF16, kind="ExternalInput")
    b1s_io = nc.dram_tensor("b1s", [128, MC], F32, kind="ExternalInput")
    out_io = nc.dram_tensor("out_loc", [SL, H], F32, kind="ExternalOutput")

    cc_in_k = nc.dram_tensor("cc_in_k", [KT_E], FP8)
    cc_out_k = nc.dram_tensor("cc_out_k", [NCORES, KT_E], FP8,
                              addr_space="Shared")
    cc_in_v = nc.dram_tensor("cc_in_v", [VT_E], FP8)
    cc_out_v = nc.dram_tensor("cc_out_v", [NCORES, VT_E], FP8,
                              addr_space="Shared")

    with tile.TileContext(nc) as tc:
        const = tc.alloc_tile_pool(name="const", bufs=1)
        persist = tc.alloc_tile_pool(name="persist", bufs=1)
        qa = tc.alloc_tile_pool(name="qa", bufs=1)
        misc = tc.alloc_tile_pool(name="misc", bufs=2)

        ident = const.tile([128, 128], BF16, name="ident", tag="ident")
        make_identity(nc, ident)
        ones_b = const.tile([1, 128], BF16, name="ones_b", tag="ones_b")
        nc.vector.memset(ones_b, 1.0)
        eps_t = const.tile([128, 1], F32, name="eps_t", tag="eps_t")
        nc.vector.memset(eps_t, EPS)
        bias_t = []
        if use_bias:
            for bi in range(5):
                bt = const.tile([1, H], BF16, name=f"bias{bi}",
                                tag=f"bias{bi}")
                nc.sync.dma_start(out=bt, in_=bias5_io[bi:bi + 1, :])
                bias_t.append(bt)
            b1s = const.tile([128, MC], F32, name="b1s", tag="b1s")
            nc.sync.dma_start(out=b1s, in_=b1s_io[:, :])

        x_sb = [persist.tile([128, H], F32, name=f"x{sb}", tag=f"x{sb}")
                for sb in range(SB)]
        x2_sb = [persist.tile([128, H], F32, name=f"x2_{sb}", tag=f"x2_{sb}")
                 for sb in range(SB)]
        qt = [qa.tile([HD, SL], BF16, name=f"qt{h}", tag=f"qt{h}")
              for h in range(NH)]
        attnT = [qa.tile([HD, SL], BF16, name=f"attnT{h}", tag=f"attnT{h}")
                 for h in range(NH)]

        # x first on the sync queue so LN1 starts as early as possible
        for sb in range(SB):
            nc.sync.dma_start(out=x_sb[sb],
                              in_=x_io[sb * 128:(sb + 1) * 128, :])

        def layernorm_bf16(src, dst):
            # dst[sb] = (src[sb] - mean) * rsqrt(var + eps), cast to bf16
            for sb in range(SB):
                stats = misc.tile([128, 5, 6], F32, name=f"lnst{sb}",
                                  tag="lnst")
                sv = src[sb].rearrange("p (g d) -> p g d", d=256)
                for g in range(5):
                    nc.vector.bn_stats(out=stats[:, g, :], in_=sv[:, g, :])
                mv = misc.tile([128, 2], F32, name=f"lnmv{sb}", tag="lnmv")
                nc.vector.bn_aggr(out=mv, in_=stats)
                rstd = misc.tile([128, 1], F32, name=f"lnrs{sb}", tag="lnrs")
                nc.scalar.activation(out=rstd, in_=mv[:, 1:2], func=AF.Sqrt,
                                     bias=eps_t)
                nc.vector.reciprocal(out=rstd, in_=rstd)
                nc.vector.tensor_scalar(out=dst[sb], in0=src[sb],
                                        scalar1=mv[:, 0:1], scalar2=rstd,
                                        op0=OP.subtract, op1=OP.mult)

        # ============== phase A: LN1, K/V/Q projections, RoPE ==========
        p_ln = tc.alloc_tile_pool(name="p_ln", bufs=1)
        p_qkv = tc.alloc_tile_pool(name="p_qkv", bufs=1)
        wpool = tc.alloc_tile_pool(name="wpool", bufs=3)
        psA_tr = tc.alloc_tile_pool(name="psA_tr", bufs=3, space="PSUM")
        psA_mm = tc.alloc_tile_pool(name="psA_mm", bufs=3, space="PSUM")

        xln = [p_ln.tile([128, H], BF16, name=f"xln{sb}", tag=f"xln{sb}")
               for sb in range(SB)]
        layernorm_bf16(x_sb, xln)
        xlnT = [p_ln.tile([128, SL], BF16, name=f"xlnT{hc}", tag=f"xlnT{hc}")
                for hc in range(HC)]
        for hc in range(HC):
            for sb in range(SB):
                pt = psA_tr.tile([128, 128], BF16, name="pt", tag="pt")
                nc.tensor.transpose(pt, xln[sb][:, hc * 128:(hc + 1) * 128],
                                    ident)
                nc.vector.tensor_copy(
                    out=xlnT[hc][:, sb * 128:(sb + 1) * 128], in_=pt)

        def load_w(w_io, name):
            w = wpool.tile([128, HC, H], BF16, name=name, tag="w")
            nc.sync.dma_start(
                out=w, in_=w_io.rearrange("(hc p) f -> p hc f", p=128))
            return w

        def project(w, bias_idx, writer):
            for sb in range(SB):
                for gi, (c0, cn) in enumerate(NCOLS):
                    ps = psA_mm.tile([128, 512], F32, name="mmps", tag="mmps")
                    for hc in range(HC):
                        nc.tensor.matmul(
                            ps[:, 0:cn],
                            lhsT=xlnT[hc][:, sb * 128:(sb + 1) * 128],
                            rhs=w[:, hc, c0:c0 + cn],
                            start=(hc == 0),
                            stop=(not use_bias and hc == HC - 1))
                    if use_bias:
                        nc.tensor.matmul(
                            ps[:, 0:cn], lhsT=ones_b,
                            rhs=bias_t[bias_idx][:, c0:c0 + cn],
                            start=False, stop=True)
                    writer(sb, gi, c0, cn, ps)

        cosr = [p_qkv.tile([128, H], BF16, name=f"cos{sb}", tag=f"cos{sb}")
                for sb in range(SB)]
        sins = [p_qkv.tile([128, H], BF16, name=f"sin{sb}", tag=f"sin{sb}")
                for sb in range(SB)]
        for sb in range(SB):
            nc.gpsimd.dma_start(out=cosr[sb],
                                in_=cos_io[sb * 128:(sb + 1) * 128, :])
            nc.gpsimd.dma_start(out=sins[sb],
                                in_=sin_io[sb * 128:(sb + 1) * 128, :])

        def rope(nat, out):
            # out = nat*cos + rotate_half(nat)*sin  (sign folded into sins)
            for sb in range(SB):
                tmp = misc.tile([128, H], BF16, name="ropetmp", tag="ropetmp")
                t3 = tmp.rearrange("p (h c) -> p h c", c=HD)
                q3 = nat[sb].rearrange("p (h c) -> p h c", c=HD)
                s3 = sins[sb].rearrange("p (h c) -> p h c", c=HD)
                nc.vector.tensor_mul(out=t3[:, :, 0:40], in0=q3[:, :, 40:80],
                                     in1=s3[:, :, 0:40])
                nc.vector.tensor_mul(out=t3[:, :, 40:80], in0=q3[:, :, 0:40],
                                     in1=s3[:, :, 40:80])
                nc.vector.tensor_mul(out=nat[sb], in0=nat[sb], in1=cosr[sb])
                nc.vector.tensor_add(out=out[sb], in0=nat[sb], in1=tmp)

        knat = [p_qkv.tile([128, H], BF16, name=f"kn{sb}", tag=f"kn{sb}")
                for sb in range(SB)]
        krope = [p_qkv.tile([128, H], BF16, name=f"kr{sb}", tag=f"kr{sb}")
                 for sb in range(SB)]
        qnat = [p_qkv.tile([128, H], BF16, name=f"qn{sb}", tag=f"qn{sb}")
                for sb in range(SB)]
        qrope = [p_qkv.tile([128, H], BF16, name=f"qr{sb}", tag=f"qr{sb}")
                 for sb in range(SB)]
        vloc = [p_qkv.tile([128, NH, HDA], FP8, name=f"vn{sb}", tag=f"vn{sb}")
                for sb in range(SB)]
        ktloc = p_qkv.tile([HD, NH, SL], FP8, name="ktloc", tag="ktloc")

        # ---- K path first: its gather gates the attention start ----
        wk = load_w(wkt_io, "wk")
        project(wk, 1, lambda sb, gi, c0, cn, ps: nc.scalar.copy(
            out=knat[sb][:, c0:c0 + cn], in_=ps[:, 0:cn]))
        rope(knat, krope)
        for h in range(NH):
            for sb in range(SB):
                ptk = psA_tr.tile([HD, 128], BF16, name="ptk", tag="pt")
                nc.tensor.transpose(ptk, krope[sb][:, h * HD:(h + 1) * HD],
                                    ident)
                nc.scalar.copy(out=ktloc[:, h, sb * 128:(sb + 1) * 128],
                               in_=ptk)
        nc.gpsimd.dma_start(
            out=cc_in_k.rearrange("(d x) -> d x", d=HD),
            in_=ktloc.rearrange("d h s -> d (h s)"))
        bar_k = nc.gpsimd.collective_compute(
            "AllGather", OP.bypass,
            replica_groups=[list(range(NCORES))],
            ins=[cc_in_k.ap()], outs=[cc_out_k.ap()])

        # ---- V path: publish in the augmented [s, h, 81] layout ----
        wv = load_w(wvt_io, "wv")
        for sb in range(SB):
            nc.vector.memset(vloc[sb][:, :, HD:HDA], 0.0)
            nc.vector.memset(vloc[sb][:, :, ZR:ZR + 1], 1.0)

        def v_writer(sb, gi, c0, cn, ps):
            h0, nh = VCOLS[gi]
            nc.scalar.copy(
                out=vloc[sb][:, h0:h0 + nh, 0:HD],
                in_=ps[:, 0:cn].rearrange("p (h d) -> p h d", d=HD))

        for sb in range(SB):
            for gi, (h0, nh) in enumerate(VCOLS):
                c0, cn = h0 * HD, nh * HD
                ps = psA_mm.tile([128, 512], F32, name="mmps", tag="mmps")
                for hc in range(HC):
                    nc.tensor.matmul(
                        ps[:, 0:cn],
                        lhsT=xlnT[hc][:, sb * 128:(sb + 1) * 128],
                        rhs=wv[:, hc, c0:c0 + cn],
                        start=(hc == 0),
                        stop=(not use_bias and hc == HC - 1))
                if use_bias:
                    nc.tensor.matmul(
                        ps[:, 0:cn], lhsT=ones_b,
                        rhs=bias_t[2][:, c0:c0 + cn],
                        start=False, stop=True)
                v_writer(sb, gi, c0, cn, ps)
        vv = cc_in_v.rearrange("(s x) -> s x", s=SL)
        for sb in range(SB):
            nc.gpsimd.dma_start(
                out=vv[sb * 128:(sb + 1) * 128, :],
                in_=vloc[sb].rearrange("p h d -> p (h d)"))
        bar_v = nc.gpsimd.collective_compute(
            "AllGather", OP.bypass,
            replica_groups=[list(range(NCORES))],
            ins=[cc_in_v.ap()], outs=[cc_out_v.ap()])

        # ---- Q path (overlaps the K gather) ----
        wq = load_w(wqt_io, "wq")
        project(wq, 0, lambda sb, gi, c0, cn, ps: nc.scalar.copy(
            out=qnat[sb][:, c0:c0 + cn], in_=ps[:, 0:cn]))
        rope(qnat, qrope)
        for h in range(NH):
            for sb in range(SB):
                ptq = psA_tr.tile([HD, 128], BF16, name="ptq", tag="pt")
                nc.tensor.transpose(ptq, qrope[sb][:, h * HD:(h + 1) * HD],
                                    ident)
                nc.vector.tensor_copy(out=qt[h][:, sb * 128:(sb + 1) * 128],
                                      in_=ptq)

        psA_mm.release()
        psA_tr.release()
        wpool.release()
        p_qkv.release()
        p_ln.release()

        # ============== phase B: attention =============================
        wop = tc.alloc_tile_pool(name="wop", bufs=1)
        w1p = tc.alloc_tile_pool(name="w1p", bufs=3)
        w2p = tc.alloc_tile_pool(name="w2p", bufs=3)
        p_att = tc.alloc_tile_pool(name="p_att", bufs=1)
        eatt = tc.alloc_tile_pool(name="eatt", bufs=2)
        natt = tc.alloc_tile_pool(name="natt", bufs=2)
        ps_sc = tc.alloc_tile_pool(name="ps_sc", bufs=3, space="PSUM")
        ps_at = tc.alloc_tile_pool(name="ps_at", bufs=2, space="PSUM")

        w1t, w2t, gtt = [], [], []

        def w1_dma(mp):
            w1 = w1p.tile([128, 2, HC, 128], BF16, name=f"w1_{mp}", tag="w1")
            nc.sync.dma_start(out=w1.rearrange("p m hc n -> p (m hc n)"),
                              in_=w1b_io[mp])
            w1t.append(w1)

        def w2_dma(mp):
            w2 = w2p.tile([128, 2, H], BF16, name=f"w2_{mp}", tag="w2")
            nc.gpsimd.dma_start(out=w2.rearrange("p m f -> p (m f)"),
                                in_=w2t_io[mp])
            w2t.append(w2)

        # gathered K^T: one tile (one contiguous DMA) per rank
        kt_r = []
        for r in range(NCORES):
            kt = p_att.tile([HD, NH, SL], FP8, name=f"ktr{r}",
                            tag=f"ktr{r}")
            kdma = nc.sync.dma_start(
                out=kt.rearrange("d h s -> d (h s)"),
                in_=cc_out_k[r].rearrange("(d x) -> d x", d=HD))
            bass_rust.add_dep_helper(kdma.ins, bar_k.ins,
                                     reason="wait for remote K gather")
            kt_r.append(kt)

        # O-proj weights and first MLP pairs stream behind the K reloads
        wo = wop.tile([HD, NH, H], BF16, name="wo", tag="wo")
        nc.sync.dma_start(
            out=wo, in_=wot_io.rearrange("(h d) f -> d h f", d=HD))
        for _mp in range(3):
            w1_dma(_mp)
            w2_dma(_mp)

        # gathered V-aug: one tile (and one DMA) per rank, 2 key blocks each
        vg2 = cc_out_v.rearrange("r (lb p x) -> r p lb x", lb=SB, p=128)
        vrk = []
        for r in range(NCORES):
            va = p_att.tile([128, SB, NH, HDA], FP8, name=f"vaug{r}",
                            tag=f"vaug{r}")
            vdma = nc.sync.dma_start(
                out=va.rearrange("p lb h d -> p lb (h d)"), in_=vg2[r])
            bass_rust.add_dep_helper(vdma.ins, bar_v.ins,
                                     reason="wait for remote V gather")
            vrk.append(va)

        e_hs = {}

        def scores_exp(h):
            e_h = eatt.tile([128, KB, SL], BF16, name=f"e{h}", tag="eh")
            e_hs[h] = e_h
            for k4 in range(4):
                ps = ps_sc.tile([128, 4 * SL], F32, name="scps", tag="scps")
                for j in range(4):
                    kb = k4 * 4 + j
                    r, lb = divmod(kb, SB)
                    nc.tensor.matmul(
                        ps[:, j * SL:(j + 1) * SL],
                        lhsT=kt_r[r][:, h, lb * 128:(lb + 1) * 128],
                        rhs=qt[h], start=True, stop=True)
                ev = e_h[:, k4 * 4:(k4 + 1) * 4, :].rearrange(
                    "p a b -> p (a b)")
                nc.scalar.activation(out=ev, in_=ps, func=AF.Exp, scale=SCALE)

        def attend(h):
            pa = ps_at.tile([HDA, SL], F32, name="atps", tag="atps")
            e_h = e_hs.pop(h)
            for kb in range(KB):
                r, lb = divmod(kb, SB)
                nc.tensor.matmul(pa,
                                 lhsT=vrk[r][:, lb, h, :],
                                 rhs=e_h[:, kb, :],
                                 start=(kb == 0), stop=(kb == KB - 1))
            # normalizer: row HD of pa holds Z[q]; reciprocal there, then
            # broadcast across partitions with a GpSimd partition bcast.
            zrow = natt.tile([HDA, SL], F32, name="zrow", tag="zrow")
            nc.vector.tensor_copy(out=zrow[ZR:ZR + 1, :], in_=pa[ZR:ZR + 1, :])
            # partition_broadcast replicates partition 0 of the tile (and
            # reciprocal misbehaves off partition 0), so hop Z down to
            # partition 0 with a tiny DMA first.
            zbc = natt.tile([1, SL], F32, name="zbc", tag="zbc")
            nc.gpsimd.dma_start(out=zbc, in_=zrow[ZR:ZR + 1, :])
            nc.vector.reciprocal_approx_fast(out=zbc, in_=zbc)
            rbs = natt.tile([HD, SL], F32, name="rbs", tag="rbs")
            nc.gpsimd.partition_broadcast(out_ap=rbs, in_ap=zbc[0:1, :])
            nc.vector.tensor_mul(out=attnT[h], in0=pa[0:HD, :], in1=rbs)

        # software pipeline: scores(h) runs on the PE while ScalarE is
        # still exponentiating head h-1 and AV(h-1) follows immediately.
        scores_exp(0)
        for h in range(1, NH):
            scores_exp(h)
            attend(h - 1)
        attend(NH - 1)

        ps_at.release()
        ps_sc.release()
        natt.release()
        eatt.release()
        p_att.release()

        # ============== phase C: O projection + residual, LN2 ==========
        p_ln2 = tc.alloc_tile_pool(name="p_ln2", bufs=1)
        psC_mm = tc.alloc_tile_pool(name="psC_mm", bufs=2, space="PSUM")
        xln2 = [p_ln2.tile([128, H], BF16, name=f"xln2{sb}", tag=f"xln2{sb}")
                for sb in range(SB)]
        xln2T = [p_ln2.tile([128, SL], BF16, name=f"xln2T{hc}",
                            tag=f"xln2T{hc}")
                 for hc in range(HC)]
        for sb in range(SB):
            for (c0, cn) in NCOLS:
                ps = psC_mm.tile([128, 512], F32, name="omm", tag="omm")
                for h in range(NH):
                    nc.tensor.matmul(ps[:, 0:cn],
                                     lhsT=attnT[h][:, sb * 128:(sb + 1) * 128],
                                     rhs=wo[:, h, c0:c0 + cn],
                                     start=(h == 0),
                                     stop=(not use_bias and h == NH - 1))
                if use_bias:
                    nc.tensor.matmul(ps[:, 0:cn], lhsT=ones_b,
                                     rhs=bias_t[3][:, c0:c0 + cn],
                                     start=False, stop=True)
                nc.vector.tensor_add(out=x2_sb[sb][:, c0:c0 + cn],
                                     in0=ps[:, 0:cn],
                                     in1=x_sb[sb][:, c0:c0 + cn])
        layernorm_bf16(x2_sb, xln2)
        for hc in range(HC):
            for sb in range(SB):
                pt = psC_mm.tile([128, 128], BF16, name="pt2", tag="omm")
                nc.tensor.transpose(pt, xln2[sb][:, hc * 128:(hc + 1) * 128],
                                    ident)
                nc.vector.tensor_copy(
                    out=xln2T[hc][:, sb * 128:(sb + 1) * 128], in_=pt)

        psC_mm.release()

        # ============== phase D: MLP stream ============================
        gtp = tc.alloc_tile_pool(name="gtp", bufs=2)
        ps_fc1 = tc.alloc_tile_pool(name="ps_fc1", bufs=2, space="PSUM")
        ps_fc2 = tc.alloc_tile_pool(name="ps_fc2", bufs=1, space="PSUM")

        fc2ps = {}
        for sb in range(SB):
            for (c0, cn) in NCOLS:
                fc2ps[(sb, c0)] = ps_fc2.tile([128, 512], F32,
                                              name=f"fc2ps{sb}_{c0}",
                                              tag=f"fc2ps{sb}_{c0}")

        def fc1(mp):
            p1 = ps_fc1.tile([128, 2, SL], F32, name="fc1ps", tag="fc1ps")
            for j in range(2):
                for hc in range(HC):
                    nc.tensor.matmul(p1[:, j, :], lhsT=w1t[mp][:, j, hc, :],
                                     rhs=xln2T[hc],
                                     start=(hc == 0), stop=(hc == HC - 1))
            gt = gtp.tile([128, 2, SL], BF16, name=f"gt{mp}", tag="gt")
            if use_bias:
                for j in range(2):
                    nc.scalar.activation(out=gt[:, j, :], in_=p1[:, j, :],
                                         func=AF.Silu, scale=1.702,
                                         bias=b1s[:, 2 * mp + j:2 * mp + j + 1])
            else:
                nc.scalar.activation(out=gt.rearrange("p a b -> p (a b)"),
                                     in_=p1.rearrange("p a b -> p (a b)"),
                                     func=AF.Silu, scale=1.702)
            gtt.append(gt)

        def fc2(mp):
            for j in range(2):
                for sb in range(SB):
                    for (c0, cn) in NCOLS:
                        nc.tensor.matmul(
                            fc2ps[(sb, c0)][:, 0:cn],
                            lhsT=gtt[mp][:, j, sb * 128:(sb + 1) * 128],
                            rhs=w2t[mp][:, j, c0:c0 + cn],
                            start=(mp == 0 and j == 0),
                            stop=(not use_bias and mp == MP - 1 and j == 1))

        for mp in range(MP):
            if mp + 3 < MP:
                w1_dma(mp + 3)
            fc1(mp)
            if mp > 0:
                fc2(mp - 1)
            if mp + 3 < MP:
                w2_dma(mp + 3)
        fc2(MP - 1)

        outp = tc.alloc_tile_pool(name="outp", bufs=1)
        outsb = [outp.tile([128, H], F32, name=f"o{sb}", tag=f"o{sb}")
                 for sb in range(SB)]
        for sb in range(SB):
            for (c0, cn) in NCOLS:
                if use_bias:
                    nc.tensor.matmul(fc2ps[(sb, c0)][:, 0:cn], lhsT=ones_b,
                                     rhs=bias_t[4][:, c0:c0 + cn],
                                     start=False, stop=True)
                nc.vector.tensor_add(out=outsb[sb][:, c0:c0 + cn],
                                     in0=fc2ps[(sb, c0)][:, 0:cn],
                                     in1=x2_sb[sb][:, c0:c0 + cn])
            nc.sync.dma_start(out=out_io[sb * 128:(sb + 1) * 128, :],
                              in_=outsb[sb])

        outp.release()
        ps_fc2.release()
        ps_fc1.release()
        gtp.release()
        p_ln2.release()
        w2p.release()
        w1p.release()
        wop.release()
        misc.release()
        qa.release()
        persist.release()
        const.release()

    nc.compile()
    return nc


_NC = {}


def _get_nc(use_bias=False):
    if use_bias not in _NC:
        _NC[use_bias] = _build_bass(use_bias)
    return _NC[use_bias]


def _prep_inputs(hidden_states, cos, sin,
                 ln1_g, ln1_b, ln2_g, ln2_b,
                 Wq, bq, Wk, bk, Wv, bv, Wo, bo,
                 W1, b1, W2, b2):
    f32 = np.float32
    x = np.asarray(hidden_states, f32).reshape(S, H)
    cos = np.asarray(cos, f32)
    sin = np.asarray(sin, f32)
    g1 = np.asarray(ln1_g, f32); be1 = np.asarray(ln1_b, f32)
    g2 = np.asarray(ln2_g, f32); be2 = np.asarray(ln2_b, f32)
    Wq = np.asarray(Wq, f32); Wk = np.asarray(Wk, f32); Wv = np.asarray(Wv, f32)
    Wo = np.asarray(Wo, f32); W1 = np.asarray(W1, f32); W2 = np.asarray(W2, f32)

    # fold LN1 affine into QKV, LN2 affine into fc1 (exact in fp32)
    wqt = (g1[:, None] * Wq.T).astype(BF)
    wkt = (g1[:, None] * Wk.T).astype(BF)
    wvt = (g1[:, None] * Wv.T).astype(BF)
    bq_e = np.asarray(bq, f32) + Wq @ be1
    bk_e = np.asarray(bk, f32) + Wk @ be1
    bv_e = np.asarray(bv, f32) + Wv @ be1
    wot = Wo.T.astype(BF)
    w1t = g2[:, None] * W1.T                       # [H, MLP]
    # device layout: [MP, 128p, (m, hc, n)] so each pair-load is contiguous
    w1b = np.ascontiguousarray(
        w1t.reshape(HC, 128, MP, 2, 128).transpose(2, 1, 3, 0, 4)
        .reshape(MP, 128, 2 * HC * 128)).astype(BF)
    b1_e = np.asarray(b1, f32) + W1 @ be2
    b1s = np.ascontiguousarray(
        (1.702 * b1_e).reshape(MC, 128).T).astype(f32)  # [128, MC]
    w2v = (W2.T / 1.702)                            # gelu scale folded
    w2t = np.ascontiguousarray(
        w2v.reshape(MP, 2, 128, H).transpose(0, 2, 1, 3)
        .reshape(MP, 128, 2 * H)).astype(BF)
    bias5 = np.stack([bq_e, bk_e, bv_e,
                      np.asarray(bo, f32), np.asarray(b2, f32)]).astype(BF)

    cos_rep = np.tile(cos, (1, NH)).astype(BF)      # [S, H]
    sin_sgn = np.concatenate([-sin[:, :40], sin[:, 40:]], axis=1)
    sin_rep = np.tile(sin_sgn, (1, NH)).astype(BF)  # [S, H]

    shared = {
        "wqt": wqt, "wkt": wkt, "wvt": wvt, "wot": wot,
        "w1b": w1b, "w2t": w2t, "bias5": bias5, "b1s": b1s,
    }
    in_maps = []
    for c in range(NCORES):
        sl = slice(c * SL, (c + 1) * SL)
        m = dict(shared)
        m["x_loc"] = np.ascontiguousarray(x[sl])
        m["cosr"] = np.ascontiguousarray(cos_rep[sl])
        m["sins"] = np.ascontiguousarray(sin_rep[sl])
        in_maps.append(m)
    return in_maps


def kernel(hidden_states, attention_mask, cos, sin,
           ln1_g, ln1_b, ln2_g, ln2_b,
           Wq, bq, Wk, bk, Wv, bv, Wo, bo,
           W1, b1, W2, b2):
    # attention_mask is all-True for this problem (spec fill: ones); the
    # dense softmax below assumes it.
    from concourse.bass_utils import run_bass_kernel_spmd

    use_bias = any(
        float(np.abs(np.asarray(b, np.float32)).max()) != 0.0
        for b in (bq, bk, bv, bo, b2))
    nc = _get_nc(use_bias)
    in_maps = _prep_inputs(hidden_states, cos, sin,
                           ln1_g, ln1_b, ln2_g, ln2_b,
                           Wq, bq, Wk, bk, Wv, bv, Wo, bo,
                           W1, b1, W2, b2)
    res = run_bass_kernel_spmd(nc, in_maps, core_ids=list(range(NCORES)))
    out = np.concatenate([res.results[c]["out_loc"] for c in range(NCORES)],
                         axis=0)
    return out.reshape(B, S, H).astype(np.float32)
